# revision 23
# baseline (speedup 1.0000x reference)
"""GCN mean-aggregation (DGL copy_src -> mean by dst) on 8 NeuronCores.

Strategy (dst-sharded, no collectives; ~3.4x over the f32/2-queue baseline):
  - Host: edges are assigned to the core owning their dst row (core c owns
    rows [c*12500, (c+1)*12500)).  Within a core, dst nodes form 98 buckets
    of 128; src rows are split into 4 groups of 25000 so gather indices fit
    int16 (dma_gather requirement).  Edges are sorted by
    (bucket-wave, src-group, bucket, src) and each (bucket, group) run is
    padded to a static number of 128-edge tiles (max over the 8 cores), so a
    single program serves all cores.  Pad edges gather a garbage row and are
    masked by dst_local = 128 (one-hot row of zeros).  Mean reciprocals
    (1/max(indeg,1)) are computed on host and shipped as a tiny table.
  - Device (identical program per core):
      * embeddings table is fp16 at 256B row stride; gathers fetch 128B
        (64 fp16) per edge via 4 SWDGE queues.  The gather is DMA
        transaction-limited (~2.2ns/idx), which sets the kernel's floor.
      * idx table DMA'd in per-wave chunks so wave-0 gathers start early
      * per bucket: ALL its one-hots built in ONE batched DVE
        tensor_tensor (fp16 iota vs broadcast dst_local, is_equal)
      * per edge-tile: psum[128,64] += onehot^T @ msgs   (fp16 matmul)
      * per bucket: scalar engine multiplies psum by rec[:, b] during the
        PSUM->SBUF copy; Sync DMAs out 128 rows
  - Host: concatenate the 8 per-core [12500, 64] outputs.
"""

import sys
from contextlib import ExitStack

import numpy as np

sys.path.insert(0, "/opt/trn_rl_repo")

import concourse.bass as bass  # noqa: E402
import concourse.mybir as mybir  # noqa: E402
import concourse.tile as tile  # noqa: E402
from concourse import bacc  # noqa: E402
from concourse.bass_utils import run_bass_kernel_spmd  # noqa: E402


def _raw_gather(gp, out_ap, in_ap, idxs_ap, num_idxs, elem_size, elem_step,
                queue_num):
    """dma_gather with elem_size_bytes below 256 (ucode supports it for
    non-transpose; the bass-level 256B assert is a transpose restriction).
    Table rows must still sit at a 256B-multiple stride (elem_step)."""
    assert idxs_ap.dtype == mybir.dt.int16
    assert in_ap.dtype == out_ap.dtype
    stride_bytes = elem_step * mybir.dt.size(in_ap.dtype)
    assert stride_bytes % 256 == 0
    inst = gp.add_instruction(
        mybir.InstDMAGatherAnt(
            name=gp.bass.get_next_instruction_name(),
            ins=[
                *gp.lower_ap_dma(in_ap, for_custom_bir_dma=True),
                gp.lower_ap(idxs_ap),
                gp.lower_val_access(gp.to_reg(num_idxs)),
            ],
            outs=[gp.lower_ap(out_ap)],
            transpose=False,
            num_idxs=num_idxs,
            elem_size=elem_size,
            stride_bytes_256=stride_bytes // 256,
            gen_mode=0,
            single_packet=True,
            queue_num=queue_num,
            sbuf_tokens_per_rank=0,
            sbuf_free_dim_per_rank=0,
            sbuf_free_dim_pad_per_rank=0,
            sbuf_byte_offset=0,
        )
    )
    return inst

N_NODES = 100000
N_EDGES = 1000000
D_FEAT = 64
N_CORES = 8
NODES_PER_CORE = N_NODES // N_CORES  # 12500
BUCKET = 128  # dst nodes per psum bucket (= one-hot free dim)
N_GROUPS = 4  # src-row groups (int16 index range for dma_gather)
WAVE = 16  # nominal buckets per gather wave (final waves taper down)


def _wave_sizes(nb, wave):
    """Front-loaded wave sizes; tapered tail shrinks the end-of-kernel
    compute backlog (last wave's compute can only start after its gathers)."""
    taper = [8, 4, 3, 2, 1]
    head = nb - sum(taper)
    sizes = [wave] * (head // wave)
    if head % wave:
        sizes.append(head % wave)
    sizes += taper
    assert sum(sizes) == nb
    return sizes
ROW = 128  # fp16 row stride in the padded table (256B)
N_QUEUES = 4


def _schedule(cnt_max, npc, bucket, wave):
    """Static schedule from per-(bucket, group) max edge counts.

    cnt_max: [nb, ngroups] max edge count over cores.
    Returns dict with tiles-per-region, waves, per-call and per-bucket info.
    """
    nb, ngroups = cnt_max.shape
    tbg = -(-cnt_max // 128)  # [nb, ngroups]
    for b in range(nb):
        if tbg[b].sum() == 0:
            tbg[b, 0] = 1  # ensure psum gets reset even for empty buckets

    sizes = _wave_sizes(nb, wave)
    bounds = np.concatenate([[0], np.cumsum(sizes)])
    waves = [range(int(bounds[i]), int(bounds[i + 1])) for i in range(len(sizes))]
    # region order: (wave, group, bucket-in-wave)
    region_tile0 = np.zeros((nb, ngroups), np.int64)
    calls = []  # [wave][group] -> (tile0, ntiles)
    t = 0
    for wv in waves:
        wcalls = []
        for g in range(ngroups):
            c0 = t
            for b in wv:
                region_tile0[b, g] = t
                t += int(tbg[b, g])
            wcalls.append((c0, t - c0))
        calls.append(wcalls)
    nt = t
    return {
        "tbg": tbg,
        "waves": waves,
        "region_tile0": region_tile0,
        "calls": calls,
        "nt": nt,
    }


def _prep(src, dst, n_nodes, n_cores, npc, bucket, ngroups, wave):
    """Sort/group/pad edges; build per-core device inputs + static schedule."""
    src = np.asarray(src, dtype=np.int64)
    dst = np.asarray(dst, dtype=np.int64)
    gsz = n_nodes // ngroups
    nb = -(-npc // bucket)

    core = dst // npc
    b = (dst - core * npc) // bucket
    g = src // gsz
    sizes = _wave_sizes(nb, wave)
    wave_of = np.repeat(np.arange(len(sizes)), sizes)
    w = wave_of[b]
    nw = len(sizes)

    cnt = np.zeros((n_cores, nb, ngroups), np.int64)
    np.add.at(cnt, (core, b, g), 1)
    sched = _schedule(cnt.max(axis=0), npc, bucket, wave)
    tbg, region_tile0, nt = sched["tbg"], sched["region_tile0"], sched["nt"]
    nslot = nt * 128

    # global sort by (core, wave, group, bucket, src)
    key = (((core * nw + w) * ngroups + g) * nb + b)
    order = np.lexsort((src, key))
    ss, ks = src[order], key[order]
    dl = (dst - (core * npc + b * bucket))[order]  # dst_local in [0, bucket)
    gs_sorted = g[order]

    kcnt = np.bincount(ks, minlength=n_cores * nw * ngroups * nb)
    kstart = np.zeros(kcnt.shape[0] + 1, np.int64)
    np.cumsum(kcnt, out=kstart[1:])
    rank = np.arange(ss.shape[0], dtype=np.int64) - kstart[ks]

    slot_base = region_tile0 * 128  # [nb, ngroups], within-core slot offset
    bo, go, co = b[order], gs_sorted, core[order]
    pos = co * nslot + slot_base[bo, go] + rank

    # per-slot group id (for pad values), same for every core
    slot_group = np.zeros(nslot, np.int64)
    for bb in range(nb):
        for gg in range(ngroups):
            t0 = region_tile0[bb, gg] * 128
            slot_group[t0 : t0 + tbg[bb, gg] * 128] = gg

    src_slot = np.tile((slot_group + 1) * gsz - 1, n_cores)  # pad: last row of group
    dstloc = np.full(n_cores * nslot, float(bucket), np.float32)
    src_slot[pos] = ss
    dstloc[pos] = dl.astype(np.float32)

    idx16 = (src_slot - np.tile(slot_group * gsz, n_cores)).astype(np.int16)
    # wrapped index layout: idx j -> partition j%16, col j//16 (x8 replicas)
    idx16 = idx16.reshape(n_cores, nt * 8, 16)
    idxtab = np.ascontiguousarray(idx16.transpose(0, 2, 1))  # [C, 16, nt*8]
    idxtab = np.tile(idxtab, (1, 8, 1))  # [C, 128, nt*8]

    # dst table columns permuted to bucket-major pass order so each bucket's
    # one-hots build in one batched tensor_tensor: perm[j] = tile id of the
    # j-th pass (bucket-major, group order within bucket).
    perm = np.concatenate(
        [
            np.concatenate(
                [
                    np.arange(region_tile0[bb, gg], region_tile0[bb, gg] + tbg[bb, gg])
                    for gg in range(ngroups)
                ]
            )
            for bb in range(nb)
        ]
    )
    dst_t = np.ascontiguousarray(
        dstloc.reshape(n_cores, nt, 128)[:, perm].transpose(0, 2, 1)
    ).astype(np.float16)  # [C, 128, nt], bucket-major
    pass0 = np.zeros(nb + 1, np.int64)
    np.cumsum(tbg.sum(axis=1), out=pass0[1:])

    # host-side mean reciprocals: rec[c, r, b] = 1/max(indeg, 1)
    indeg = np.bincount(dst, minlength=n_nodes).astype(np.float32)
    rec_rows = 1.0 / np.maximum(indeg, 1.0)
    rec = np.ones((n_cores, nb * bucket), np.float32)
    rec[:, :npc] = rec_rows.reshape(n_cores, npc)
    rec = np.ascontiguousarray(
        rec.reshape(n_cores, nb, bucket).transpose(0, 2, 1)
    )  # [C, 128, nb]
    sched["pass0"] = pass0
    return idxtab, dst_t, rec, sched


def _build(n_nodes, d_feat, npc, bucket, ngroups, sched):
    """Build the (per-core) Bass program."""
    gsz = n_nodes // ngroups
    nb = -(-npc // bucket)
    nt = sched["nt"]
    tbg, region_tile0 = sched["tbg"], sched["region_tile0"]
    f32 = mybir.dt.float32
    f16 = mybir.dt.float16
    i16 = mybir.dt.int16

    pass0 = sched["pass0"]
    kmax = int(max(pass0[b + 1] - pass0[b] for b in range(nb)))

    nc = bacc.Bacc(
        "TRN2", target_bir_lowering=False, debug=False,
        num_swdge_queues=N_QUEUES,
    )
    emb = nc.dram_tensor("emb", [n_nodes, ROW], f16, kind="ExternalInput")
    idx_t = nc.dram_tensor("idx_t", [128, nt * 8], i16, kind="ExternalInput")
    dst_t = nc.dram_tensor("dst_t", [128, nt], f16, kind="ExternalInput")
    rec_t = nc.dram_tensor("rec_t", [128, nb], f32, kind="ExternalInput")
    iota_t = nc.dram_tensor("iota_t", [128, kmax * bucket], f16, kind="ExternalInput")
    out = nc.dram_tensor("out", [npc, d_feat], f32, kind="ExternalOutput")

    with tile.TileContext(nc) as tc, ExitStack() as ctx:
        const_p = ctx.enter_context(tc.tile_pool(name="const", bufs=1))
        idx_p = ctx.enter_context(tc.tile_pool(name="idx", bufs=1))
        msgs_p = ctx.enter_context(tc.tile_pool(name="msgs", bufs=5))
        oh_p = ctx.enter_context(tc.tile_pool(name="oh", bufs=4))
        ps_p = ctx.enter_context(tc.tile_pool(name="ps", bufs=8, space="PSUM"))
        outp_p = ctx.enter_context(tc.tile_pool(name="outp", bufs=4))

        idxall = idx_p.tile([128, nt * 8], i16)
        # chunked so wave-0 gathers start without waiting for the full table;
        # wave 0 split per group (first gather gates on ~1/28th of the table)
        wb = [sched["calls"][wvi][0][0] for wvi in range(len(sched["waves"]))] + [nt]
        chunks = []
        first = True
        for gg in range(ngroups):
            t0, ntl = sched["calls"][0][gg]
            if ntl:
                if first and ntl > 8:
                    chunks.append((t0, t0 + 8))
                    chunks.append((t0 + 8, t0 + ntl))
                    first = False
                else:
                    chunks.append((t0, t0 + ntl))
        for wvi in range(1, len(sched["waves"])):
            chunks.append((wb[wvi], wb[wvi + 1]))
        for t0, t1 in chunks:
            nc.sync.dma_start(out=idxall[:, t0 * 8 : t1 * 8], in_=idx_t[:, t0 * 8 : t1 * 8])
        dstall = idx_p.tile([128, nt], f16)
        nc.sync.dma_start(out=dstall[:], in_=dst_t[:, :])
        recall = idx_p.tile([128, nb], f32)
        nc.sync.dma_start(out=recall[:], in_=rec_t[:, :])

        iota_k = const_p.tile([128, kmax * bucket], f16)
        nc.sync.dma_start(out=iota_k[:], in_=iota_t[:, :])

        qn = 0
        for wvi, wv in enumerate(sched["waves"]):
            msgs = {}
            call0 = {}
            for gg in range(ngroups):
                t0, ntl = sched["calls"][wvi][gg]
                call0[gg] = t0
                if ntl == 0:
                    continue
                m = msgs_p.tile([128, ntl * d_feat], f16, tag=f"msgs{gg}")
                msgs[gg] = m
                # dma_gather is limited to 1024 indices (8 tiles) per call
                for sc in range(0, ntl, 8):
                    k = min(8, ntl - sc)
                    ts = t0 + sc
                    _raw_gather(
                        nc.gpsimd,
                        out_ap=m[:, sc * d_feat : (sc + k) * d_feat].rearrange(
                            "p (t e) -> p t e", e=d_feat
                        ),
                        in_ap=emb[gg * gsz : (gg + 1) * gsz, :d_feat],
                        idxs_ap=idxall[:, ts * 8 : (ts + k) * 8],
                        num_idxs=k * 128,
                        elem_size=d_feat,
                        elem_step=ROW,
                        queue_num=qn,
                    )
                    qn = (qn + 1) % N_QUEUES
            for bb in wv:
                passes = [
                    (gg, region_tile0[bb, gg] + j)
                    for gg in range(ngroups)
                    for j in range(int(tbg[bb, gg]))
                ]
                np_ = len(passes)
                p0 = int(pass0[bb])
                # all one-hots of the bucket in one batched DVE instruction
                oh = oh_p.tile([128, kmax * bucket], f16)
                nc.vector.tensor_tensor(
                    out=oh[:, : np_ * bucket].rearrange("p (k e) -> p k e", e=bucket),
                    in0=iota_k[:, : np_ * bucket].rearrange("p (k e) -> p k e", e=bucket),
                    in1=dstall[:, p0 : p0 + np_].to_broadcast([128, np_, bucket]),
                    op=mybir.AluOpType.is_equal,
                )
                psum = ps_p.tile([bucket, d_feat], f32)
                for i, (gg, t) in enumerate(passes):
                    off = int(t - call0[gg]) * d_feat
                    nc.tensor.matmul(
                        out=psum[:],
                        lhsT=oh[:, i * bucket : (i + 1) * bucket],
                        rhs=msgs[gg][:, off : off + d_feat],
                        start=(i == 0),
                        stop=(i == len(passes) - 1),
                    )
                nrows = min(bucket, npc - bb * bucket)
                ot = outp_p.tile([bucket, d_feat], f32)
                nc.scalar.mul(ot[:], psum[:], recall[:, bb : bb + 1])
                nc.sync.dma_start(
                    out=out[bb * bucket : bb * bucket + nrows, :], in_=ot[:nrows, :]
                )

    nc.compile()
    return nc


_CACHE = {}


def _run(embeddings, src, dst, trace=False, trace_kwargs=None):
    embeddings = np.asarray(embeddings, dtype=np.float32)
    embh = np.zeros((N_NODES, ROW), np.float16)
    embh[:, :D_FEAT] = embeddings.astype(np.float16)
    idxtab, dst_t, rec, sched = _prep(
        src, dst, N_NODES, N_CORES, NODES_PER_CORE, BUCKET, N_GROUPS, WAVE
    )
    key = sched["tbg"].tobytes()
    if key not in _CACHE:
        _CACHE[key] = _build(N_NODES, D_FEAT, NODES_PER_CORE, BUCKET, N_GROUPS, sched)
    nc = _CACHE[key]

    pass0 = sched["pass0"]
    kmax = int(max(pass0[b + 1] - pass0[b] for b in range(len(pass0) - 1)))
    iota_tab = np.tile(np.arange(BUCKET, dtype=np.float16), (128, kmax))
    in_maps = [
        {
            "emb": embh,
            "idx_t": idxtab[c],
            "dst_t": dst_t[c],
            "rec_t": rec[c],
            "iota_t": iota_tab,
        }
        for c in range(N_CORES)
    ]
    res = run_bass_kernel_spmd(
        nc,
        in_maps,
        core_ids=list(range(N_CORES)),
        trace=trace,
        **(trace_kwargs or {}),
    )
    out = np.concatenate([res.results[c]["out"] for c in range(N_CORES)], axis=0)
    return out, res


def kernel(embeddings, src, dst):
    out, _ = _run(embeddings, src, dst, trace=False)
    return out
